# revision 3
# baseline (speedup 1.0000x reference)
"""Trainium2 Bass kernel for nn_NestedAttention (B=16, C=512, W=H=32).

Data-parallel over batch: 8 NeuronCores x 2 batch elements each.
Per batch element (all on one core):
    pooled = mean_spatial(x); w = sigmoid(fc2(gelu(fc1(pooled))))  [SE gate]
    q = Wq x + bq ; k = Wk x + bk ; v0 = Wv x            (1x1 convs, N=1024)
    logits = q^T k + pos^T q      (pos = rel_h + rel_w flattened, [C,N])
    attn   = softmax_rows(logits)
    out    = (v0 attn^T + bv) * w^2     (bv folded: rows of attn sum to 1)

Matmuls run as float32r (full-rate fp32 on the PE when free dim >= 256).
Weights are pre-transposed on the host so every matmul operand is in its
natural [K(contraction) x free] SBUF layout.
"""

import math
from contextlib import ExitStack

import numpy as np

import concourse.bass as bass
import concourse.tile as tile
from concourse import bacc, masks, mybir
from concourse.bass_utils import run_bass_kernel_spmd

B, C, W, H = 16, 512, 32, 32
N = W * H                  # 1024 spatial positions
CH = C // 16               # 32 SE hidden units
NCORES = 8
BPC = B // NCORES          # 2 batch elements per core
CB = C // 128              # 4 channel blocks
NB = N // 128              # 8 spatial blocks
F32 = mybir.dt.float32
F32R = mybir.dt.float32r

USE_F32R = True
MMDT = F32R if USE_F32R else F32   # dtype for all big-matmul operands


def _mm(nc, out, lhsT, rhs, start, stop):
    nc.tensor.matmul(out, lhsT, rhs, start=start, stop=stop)


def build(reps: int = 1):
    """Build + compile the per-core Bass program. Returns the Bacc object."""
    nc = bacc.Bacc("TRN2", target_bir_lowering=False, debug=False, num_devices=1)

    x_d = nc.dram_tensor("x", [BPC, C, N], MMDT, kind="ExternalInput")
    wqT_d = nc.dram_tensor("wqT", [C, C], MMDT, kind="ExternalInput")
    wkT_d = nc.dram_tensor("wkT", [C, C], MMDT, kind="ExternalInput")
    wvT_d = nc.dram_tensor("wvT", [C, C], MMDT, kind="ExternalInput")
    bq_d = nc.dram_tensor("bq", [C], F32, kind="ExternalInput")
    bk_d = nc.dram_tensor("bk", [C], F32, kind="ExternalInput")
    bv_d = nc.dram_tensor("bv", [C], F32, kind="ExternalInput")
    pos_d = nc.dram_tensor("pos", [C, N], MMDT, kind="ExternalInput")
    fc1T_d = nc.dram_tensor("fc1T", [C, CH], F32, kind="ExternalInput")
    fc2T_d = nc.dram_tensor("fc2T", [CH, C], F32, kind="ExternalInput")
    y_d = nc.dram_tensor("y", [BPC, C, N], F32, kind="ExternalOutput")

    with tile.TileContext(nc) as tc, ExitStack() as ctx:
        _body(ctx, tc, x_d, wqT_d, wkT_d, wvT_d, bq_d, bk_d, bv_d, pos_d,
              fc1T_d, fc2T_d, y_d, reps)

    nc.compile()
    return nc


def _body(ctx, tc, x_d, wqT_d, wkT_d, wvT_d, bq_d, bk_d, bv_d, pos_d,
          fc1T_d, fc2T_d, y_d, reps):
    nc = tc.nc
    AF = mybir.ActivationFunctionType

    const = ctx.enter_context(tc.tile_pool(name="const", bufs=1))
    xpool = ctx.enter_context(tc.tile_pool(name="x", bufs=2))
    work = ctx.enter_context(tc.tile_pool(name="work", bufs=1))
    expp = ctx.enter_context(tc.tile_pool(name="expp", bufs=2))
    ypool = ctx.enter_context(tc.tile_pool(name="y", bufs=2))
    stat = ctx.enter_context(tc.tile_pool(name="stat", bufs=4))
    ps_big = ctx.enter_context(tc.tile_pool(name="ps_big", bufs=2, space="PSUM"))
    ps_sm = ctx.enter_context(tc.tile_pool(name="ps_sm", bufs=2, space="PSUM"))
    ps_se = ctx.enter_context(tc.tile_pool(name="ps_se", bufs=2, space="PSUM"))

    # ---- constants (loaded once) ----
    ident_f = const.tile([128, 128], F32)
    masks.make_identity(nc, ident_f[:])
    ident = ident_f
    if MMDT is not F32:
        ident = const.tile([128, 128], MMDT)
        nc.vector.tensor_copy(ident[:], ident_f[:])

    wq_sb = const.tile([128, CB * C], MMDT)   # chunk ci at [:, ci*512:+512] = WqT[ci]
    wk_sb = const.tile([128, CB * C], MMDT)
    wv_sb = const.tile([128, CB * C], MMDT)
    for w_sb, w_d in ((wq_sb, wqT_d), (wk_sb, wkT_d), (wv_sb, wvT_d)):
        for ci in range(CB):
            nc.sync.dma_start(w_sb[:, ci * C:(ci + 1) * C],
                              w_d.ap()[ci * 128:(ci + 1) * 128, :])

    pos_sb = const.tile([128, CB * N], MMDT)  # chunk ci at [:, ci*1024:+1024]
    for ci in range(CB):
        nc.sync.dma_start(pos_sb[:, ci * N:(ci + 1) * N],
                          pos_d.ap()[ci * 128:(ci + 1) * 128, :])

    bq_sb = const.tile([128, CB], F32)
    bk_sb = const.tile([128, CB], F32)
    bv_sb = const.tile([128, CB], F32)
    for b_sb, b_d in ((bq_sb, bq_d), (bk_sb, bk_d), (bv_sb, bv_d)):
        nc.sync.dma_start(b_sb[:], b_d.ap().rearrange("(cb p) -> p cb", p=128))

    fc1_sb = const.tile([128, CB * CH], F32)  # chunk ci at [:, ci*32:+32]
    for ci in range(CB):
        nc.sync.dma_start(fc1_sb[:, ci * CH:(ci + 1) * CH],
                          fc1T_d.ap()[ci * 128:(ci + 1) * 128, :])
    fc2_sb = const.tile([CH, C], F32)
    nc.sync.dma_start(fc2_sb[:], fc2T_d.ap())

    for _ in range(reps):
        # ---- load x for both batch elements ----
        xb = []
        for b in range(BPC):
            xt = xpool.tile([128, CB * N], MMDT, tag="x")
            for ci in range(CB):
                nc.sync.dma_start(xt[:, ci * N:(ci + 1) * N],
                                  x_d.ap()[b, ci * 128:(ci + 1) * 128, :])
            xb.append(xt)

        # ---- SE gate for both batches first (one sigmoid/erf table set) ----
        w2 = const.tile([128, BPC * CB], F32, tag="w2")     # sigmoid^2 per (b, c)
        bvw2 = const.tile([128, BPC * CB], F32, tag="bvw2")  # bv * w2 per (b, c)
        for b in range(BPC):
            pooled = stat.tile([128, CB], F32, tag="pooled")
            for ci in range(CB):
                nc.vector.reduce_sum(pooled[:, ci:ci + 1],
                                     xb[b][:, ci * N:(ci + 1) * N].bitcast(F32),
                                     axis=mybir.AxisListType.X)
            ph = ps_se.tile([128, 1], F32, tag="se")
            for ci in range(CB):
                nc.tensor.matmul(ph[0:CH, :], fc1_sb[:, ci * CH:(ci + 1) * CH],
                                 pooled[:, ci:ci + 1],
                                 start=(ci == 0), stop=(ci == CB - 1))
            # exact gelu(p) = 0.5 p (1 + erf(p/sqrt(2))); p = pooled @ fc1T
            hp = stat.tile([128, 1], F32, tag="hp")      # p (mean-scaled)
            er = stat.tile([128, 1], F32, tag="er")
            hid = stat.tile([128, 1], F32, tag="hid")
            nc.scalar.activation(hp[0:CH, :], ph[0:CH, :], AF.Copy, scale=1.0 / N)
            nc.scalar.activation(er[0:CH, :], hp[0:CH, :], AF.Erf,
                                 scale=1.0 / math.sqrt(2.0))
            nc.vector.tensor_scalar_add(er[0:CH, :], er[0:CH, :], 1.0)
            nc.vector.tensor_mul(hid[0:CH, :], hp[0:CH, :], er[0:CH, :])
            nc.vector.tensor_scalar_mul(hid[0:CH, :], hid[0:CH, :], 0.5)
            for co in range(CB):
                pw = ps_se.tile([128, 1], F32, tag="se")
                nc.tensor.matmul(pw[:], fc2_sb[:, co * 128:(co + 1) * 128],
                                 hid[0:CH, :], start=True, stop=True)
                col = b * CB + co
                wcol = stat.tile([128, 1], F32, tag="wcol")
                nc.scalar.activation(wcol[:], pw[:], AF.Sigmoid)
                nc.vector.tensor_mul(w2[:, col:col + 1], wcol[:], wcol[:])
                nc.vector.tensor_mul(bvw2[:, col:col + 1], bv_sb[:, co:co + 1],
                                     w2[:, col:col + 1])

        # ---- main attention per batch ----
        for b in range(BPC):
            # q = WqT^T x + bq, k likewise (layout [c, n]); vT = x^T WvT ([m, c])
            q_sb = work.tile([128, CB * N], MMDT, tag="q")
            k_sb = work.tile([128, CB * N], MMDT, tag="k")
            vT_sb = work.tile([128, NB * C], MMDT, tag="vT")
            for dst, w_sb, b_sb in ((q_sb, wq_sb, bq_sb), (k_sb, wk_sb, bk_sb)):
                for co in range(CB):
                    pq = ps_big.tile([128, N], F32, tag="big")
                    for nh in range(2):
                        for ci in range(CB):
                            _mm(nc, pq[:, nh * 512:(nh + 1) * 512],
                                w_sb[:, ci * C + co * 128: ci * C + co * 128 + 128],
                                xb[b][:, ci * N + nh * 512: ci * N + (nh + 1) * 512],
                                start=(ci == 0), stop=(ci == CB - 1))
                    nc.scalar.activation(dst[:, co * N:(co + 1) * N], pq[:],
                                         AF.Identity, bias=b_sb[:, co:co + 1])
            for mb in range(NB):
                pv = ps_sm.tile([128, 512], F32, tag="sm")
                for ci in range(CB):
                    _mm(nc, pv[:],
                        xb[b][:, ci * N + mb * 128: ci * N + mb * 128 + 128],
                        wv_sb[:, ci * C:(ci + 1) * C],
                        start=(ci == 0), stop=(ci == CB - 1))
                nc.vector.tensor_copy(vT_sb[:, mb * C:(mb + 1) * C], pv[:])

            # logits -> softmax -> transposed attention, per 128-row block
            attnT = work.tile([128, NB * N], MMDT, tag="attnT")
            attnT_v = attnT[:].rearrange("p (mb n) -> p mb n", n=N)
            for nb in range(NB):
                pl = ps_big.tile([128, N], F32, tag="big")
                for mh in range(2):
                    dst = pl[:, mh * 512:(mh + 1) * 512]
                    for ci in range(CB):
                        _mm(nc, dst,
                            q_sb[:, ci * N + nb * 128: ci * N + nb * 128 + 128],
                            k_sb[:, ci * N + mh * 512: ci * N + (mh + 1) * 512],
                            start=(ci == 0), stop=False)
                    for ci in range(CB):
                        _mm(nc, dst,
                            pos_sb[:, ci * N + nb * 128: ci * N + nb * 128 + 128],
                            q_sb[:, ci * N + mh * 512: ci * N + (mh + 1) * 512],
                            start=False, stop=(ci == CB - 1))
                mx = stat.tile([128, 1], F32, tag="mx")
                nc.vector.reduce_max(mx[:], pl[:], axis=mybir.AxisListType.X,
                                     negate=True)
                ex = expp.tile([128, N], MMDT, tag="exp")
                rs = stat.tile([128, 1], F32, tag="rs")
                nc.scalar.activation(ex[:], pl[:], AF.Exp, bias=mx[:],
                                     accum_out=rs[:])
                ri = stat.tile([128, 1], F32, tag="ri")
                nc.vector.reciprocal(ri[:], rs[:])
                nc.vector.tensor_scalar_mul(ex[:], ex[:], ri[:])
                for mg in range(2):
                    pt = ps_sm.tile([128, 512], MMDT, tag="sm")
                    for j in range(4):
                        mb = mg * 4 + j
                        nc.tensor.transpose(pt[:, j * 128:(j + 1) * 128],
                                            ex[:, mb * 128:(mb + 1) * 128],
                                            ident[:])
                    nc.vector.tensor_copy(
                        attnT_v[:, mg * 4:(mg + 1) * 4, nb * 128:(nb + 1) * 128],
                        pt[:].rearrange("p (j n) -> p j n", n=128))

            # out[c, n] = sum_m v0[c, m] attn[n, m]; then *w2 + bv*w2
            for co in range(CB):
                po = ps_big.tile([128, N], F32, tag="big")
                for nh in range(2):
                    for mb in range(NB):
                        _mm(nc, po[:, nh * 512:(nh + 1) * 512],
                            vT_sb[:, mb * C + co * 128: mb * C + co * 128 + 128],
                            attnT[:, mb * N + nh * 512: mb * N + (nh + 1) * 512],
                            start=(mb == 0), stop=(mb == NB - 1))
                col = b * CB + co
                yt = ypool.tile([128, N], F32, tag="y")
                nc.scalar.activation(yt[:], po[:], AF.Identity,
                                     bias=bvw2[:, col:col + 1],
                                     scale=w2[:, col:col + 1])
                nc.sync.dma_start(y_d.ap()[b, co * 128:(co + 1) * 128, :], yt[:])


def prep_inputs(x, Wq, bq, Wk, bk, Wv, bv, rel_h, rel_w, Wfc1, Wfc2):
    """Host-side marshaling: per-core input dicts (batch-sharded x, shared
    pre-transposed weights)."""
    x = np.ascontiguousarray(np.asarray(x, dtype=np.float32).reshape(B, C, N))
    pos = (np.asarray(rel_h, dtype=np.float32) +
           np.asarray(rel_w, dtype=np.float32)).reshape(C, N)
    shared = {
        "wqT": np.ascontiguousarray(np.asarray(Wq, np.float32).T),
        "wkT": np.ascontiguousarray(np.asarray(Wk, np.float32).T),
        "wvT": np.ascontiguousarray(np.asarray(Wv, np.float32).T),
        "bq": np.asarray(bq, np.float32),
        "bk": np.asarray(bk, np.float32),
        "bv": np.asarray(bv, np.float32),
        "pos": np.ascontiguousarray(pos),
        "fc1T": np.ascontiguousarray(np.asarray(Wfc1, np.float32).T),
        "fc2T": np.ascontiguousarray(np.asarray(Wfc2, np.float32).T),
    }
    return [dict(shared, x=np.ascontiguousarray(x[c * BPC:(c + 1) * BPC]))
            for c in range(NCORES)]


_NC_CACHE = {}


def _get_nc(reps=1):
    if reps not in _NC_CACHE:
        _NC_CACHE[reps] = build(reps)
    return _NC_CACHE[reps]


def kernel(**inputs) -> np.ndarray:
    nc = _get_nc()
    in_maps = prep_inputs(**inputs)
    res = run_bass_kernel_spmd(nc, in_maps, core_ids=list(range(NCORES)))
    out = np.concatenate([res.results[c]["y"] for c in range(NCORES)], axis=0)
    return out.reshape(B, C, W, H).astype(np.float32)
